# revision 14
# baseline (speedup 1.0000x reference)
"""Trainium2 Bass kernel for nn_CustomLSTM_78486232367302.

Custom LSTM with layer-normed hidden state, an energy-integration side state,
and a scalar output head.  B=256, T=1024, I=64, H=512.

Sharding: data-parallel over batch across 8 NeuronCores (32 rows each);
weights replicated.  The recurrence is sequential in T.

Layout strategy per core (batch Bc=32):
  - "resh" layout for all elementwise state: [128, 128] where partition
    p = 32*q + b (q = H-quarter 0..3, b = batch row) and free dim = H index
    within quarter.  This keeps all 128 DVE/ACT lanes busy.
  - Gate matmuls run column-tiled (128x32 PE mode): tile j computes
    [i_j, f_j, g_j, o_j] (H-quarter j of each gate) for all 32 batch rows,
    accumulating over 9 K-chunks of the concatenated input
    [x_t (64) + bias row | t_en (512) | h_norm (512)].  Output PSUM is
    directly in resh layout: [128, 512] = (quarter,batch) x [i|f|g|o].
  - Gate biases and the LN affine of h_norm are folded into the weight
    matrix / an extra all-ones contraction row (host side).
  - Per-step PE transposes convert resh-layout tensors to the transposed
    [K, 32] stationary layout needed for the next step's matmuls.
"""

import os
import sys

sys.path.insert(0, "/opt/trn_rl_repo")

import numpy as np

B, T, I, H = 256, 1024, 64, 512
N_CORES = 8
BC = B // N_CORES          # 32 batch rows per core
Q = 4                      # H quarters
HQ = H // Q                # 128
D_GATE = 4 * H             # 2048 gate outputs
EPS = 1e-5

_PROGRAM_CACHE = {}


def _resh(a):
    """[Bc, H] -> [128, HQ] resh layout: out[32q+b, m] = a[b, 128q+m]."""
    return a.reshape(BC, Q, HQ).transpose(1, 0, 2).reshape(Q * BC, HQ)


def _unresh(a):
    """[128, HQ] -> [Bc, H]."""
    return a.reshape(Q, BC, HQ).transpose(1, 0, 2).reshape(BC, H)


def _build_program(n_chunks, chunk, fast_ln, wl, repeats=1, wet_bf16=True,
                   unroll=1, stagger=False):
    """Build the bass program. Returns (nc, meta)."""
    import concourse.bacc as bacc
    import concourse.tile as tile
    import concourse.bass as bass
    from concourse import mybir

    f32 = mybir.dt.float32
    bf16 = mybir.dt.bfloat16
    AF = mybir.ActivationFunctionType
    OP = mybir.AluOpType
    AX = mybir.AxisListType

    t_total = n_chunks * chunk

    nc = bacc.Bacc("TRN2", target_bir_lowering=False, debug=False,
                   enable_asserts=False, num_devices=N_CORES)

    # ---- DRAM I/O ----
    d_xt = nc.dram_tensor("xt", [I + 1, BC * t_total], f32, kind="ExternalInput").ap()
    d_cxc = nc.dram_tensor("cxc", [128, t_total], f32, kind="ExternalInput").ap()
    d_cxp = nc.dram_tensor("cxp", [128, t_total], f32, kind="ExternalInput").ap()
    d_wm0 = nc.dram_tensor("wm0", [I + 1, D_GATE], f32, kind="ExternalInput").ap()
    d_wmr = nc.dram_tensor("wmr", [128, 8 * D_GATE], f32, kind="ExternalInput").ap()
    wet_dt = bf16 if wet_bf16 else f32
    d_wet = nc.dram_tensor("wet", [128, 2048], wet_dt, kind="ExternalInput").ap()
    d_wfcr = nc.dram_tensor("wfcr", [128, HQ], f32, kind="ExternalInput").ap()
    d_ssel = nc.dram_tensor("ssel", [128, 128], f32, kind="ExternalInput").ap()
    d_id128 = nc.dram_tensor("id128", [128, 128], f32, kind="ExternalInput").ap()
    if not fast_ln:
        d_gresh = nc.dram_tensor("gresh", [128, HQ], f32, kind="ExternalInput").ap()
        d_bresh = nc.dram_tensor("bresh", [128, HQ], f32, kind="ExternalInput").ap()

    d_outs = nc.dram_tensor("outs", [BC, t_total], f32, kind="ExternalOutput").ap()
    d_ens = nc.dram_tensor("ens", [BC, t_total], f32, kind="ExternalOutput").ap()
    d_hf = nc.dram_tensor("hf", [128, HQ], f32, kind="ExternalOutput").ap()
    d_cf = nc.dram_tensor("cf", [128, HQ], f32, kind="ExternalOutput").ap()
    d_henf = nc.dram_tensor("henf", [128, HQ], f32, kind="ExternalOutput").ap()

    with tile.TileContext(nc) as tc:
        with (
            tc.tile_pool(name="consts", bufs=1) as consts,
            tc.tile_pool(name="state", bufs=1) as state,
            tc.tile_pool(name="xbufp", bufs=2) as xbufp,
            tc.tile_pool(name="trans", bufs=2) as trans,
            tc.tile_pool(name="psum", bufs=1, space="PSUM") as psum,
        ):
            # ---- constants (DMA once) ----
            wm0 = consts.tile([I + 1, D_GATE], f32, tag="wm0")
            nc.sync.dma_start(out=wm0, in_=d_wm0)
            wmr = consts.tile([128, 8 * D_GATE], f32, tag="wmr")
            nc.sync.dma_start(out=wmr, in_=d_wmr)
            wet = consts.tile([128, 2048], wet_dt, tag="wet")
            nc.sync.dma_start(out=wet, in_=d_wet)
            wfcr = consts.tile([128, HQ], f32, tag="wfcr")
            nc.sync.dma_start(out=wfcr, in_=d_wfcr)
            ssel = consts.tile([128, 128], f32, tag="ssel")
            nc.sync.dma_start(out=ssel, in_=d_ssel)
            id128 = consts.tile([128, 128], f32, tag="id128")
            nc.sync.dma_start(out=id128, in_=d_id128)
            cxc = consts.tile([128, t_total], f32, tag="cxc")
            nc.sync.dma_start(out=cxc, in_=d_cxc)
            cxp = consts.tile([128, t_total], f32, tag="cxp")
            nc.sync.dma_start(out=cxp, in_=d_cxp)
            if not fast_ln:
                gresh = consts.tile([128, HQ], f32, tag="gresh")
                nc.sync.dma_start(out=gresh, in_=d_gresh)
                bresh = consts.tile([128, HQ], f32, tag="bresh")
                nc.sync.dma_start(out=bresh, in_=d_bresh)
            eps_t = consts.tile([128, 1], f32, tag="eps")
            nc.vector.memset(eps_t, EPS)

            # ---- persistent state tiles ----
            c_r = state.tile([128, HQ], f32, tag="c_r")
            h_r = state.tile([128, HQ], f32, tag="h_r")        # h (resh)
            hen_r = state.tile([128, HQ], f32, tag="hen_r")    # h_en (resh)
            zT = state.tile([128, 128], f32, tag="zT")         # h_norm.T chunks
            teT = state.tile([128, 128], f32, tag="teT")       # t_en.T chunks
            heT = state.tile([128, 128], wet_dt, tag="heT")    # h_en.T chunks
            outb = state.tile([128, t_total + 1], f32, tag="outb")
            enb = state.tile([128, t_total + 1], f32, tag="enb")


            # psum tiles.  NOTE: matmul start=True zeroes the whole 2KB bank
            # row for the touched partitions, so accumulation groups that
            # overlap in time must live in different banks (or disjoint
            # partitions, as the column-tiled gate matmuls do).
            ps_g = psum.tile([128, 512], f32, tag="ps_g")
            ps_te = [psum.tile([128, 32], f32, tag=f"ps_te{m}",
                               name=f"ps_te{m}")
                     for m in range(4)]
            ps_tz = psum.tile([128, 128], f32, tag="ps_tz")
            ps_th = psum.tile([128, 128], f32, tag="ps_th")
            # small psums share one bank; lifetimes are sequential per step
            ps_small = psum.tile([128, 8], f32, tag="ps_small")
            ps_s = ps_small[:, 0:2]
            ps_s2 = ps_small[:, 2:4]
            ps_w = ps_small[:, 4:5]

            def init_state():
                nc.vector.memset(c_r, 0.0)
                nc.vector.memset(h_r, 0.0)
                nc.vector.memset(hen_r, 0.0)
                nc.vector.memset(zT, 0.0)
                nc.vector.memset(teT, 0.0)
                nc.vector.memset(outb, 0.0)
                nc.vector.memset(enb, 0.0)

            def alloc_trans():
                tl = {}
                for nm, shp, dt in [
                    ("sx", [I + 1, 32], f32), ("gates", [128, 512], f32),
                    ("tmp_ig", [128, HQ], f32), ("tc_t", [128, HQ], f32),
                    ("u_t", [128, HQ], f32), ("z1", [128, HQ], f32),
                    ("z2", [128, HQ], f32), ("hsum", [128, HQ], f32),
                    ("p1", [128, HQ], f32), ("st6", [128, 6], f32),
                    ("mv", [128, 2], f32), ("stat2", [128, 2], f32),
                    ("mvf", [128, 2], f32), ("var_t", [128, 1], f32),
                    ("rstd", [128, 1], f32), ("v2", [128, 1], f32),
                    ("rstd2", [128, 1], f32), ("ve_t", [128, 1], f32),
                    ("nt_t", [128, 1], f32), ("ddr", [128, 1], f32),
                    ("cxwl", [128, 1], f32), ("so_t", [128, 1], f32),
                    ("rr", [128, 1], f32),
                ] + ([] if fast_ln else [
                    ("hnew", [128, HQ], f32), ("st6b", [128, 6], f32),
                    ("mvb", [128, 2], f32), ("stat2b", [128, 2], f32),
                    ("mvfb", [128, 2], f32), ("var2_t", [128, 1], f32),
                ]):
                    tl[nm] = trans.tile(shp, dt, tag=nm, name=f"t_{nm}")
                return tl

            def rsqrt_acc(tl, out_r, var_ap):
                """out_r = 1/sqrt(var_ap + EPS).  The ACT Sqrt table is only
                ~6e-6 accurate; one Newton step brings rsqrt to ~1e-7."""
                ve_t, nt_t = tl["ve_t"], tl["nt_t"]
                nc.vector.tensor_scalar(out=ve_t, in0=var_ap,
                                        scalar1=float(EPS), scalar2=None,
                                        op0=OP.add)
                nc.scalar.activation(out_r, ve_t, AF.Sqrt)
                nc.vector.reciprocal(out_r, out_r)
                nc.vector.tensor_scalar(out=nt_t, in0=out_r, scalar1=out_r,
                                        scalar2=None, op0=OP.mult)
                nc.vector.tensor_mul(nt_t, nt_t, ve_t)
                nc.vector.tensor_scalar(out=nt_t, in0=nt_t, scalar1=-0.5,
                                        scalar2=1.5, op0=OP.mult, op1=OP.add)
                nc.vector.tensor_scalar(out=out_r, in0=out_r, scalar1=nt_t,
                                        scalar2=None, op0=OP.mult)

            def step_body(t_g, x_off):
                """One LSTM step.  t_g = global step; x_off = step index
                within the current x chunk (either may be reg exprs)."""
                tl = alloc_trans()
                sx, gates = tl["sx"], tl["gates"]
                tmp_ig, tc_t, u_t = tl["tmp_ig"], tl["tc_t"], tl["u_t"]
                z1, z2, hsum, p1 = tl["z1"], tl["z2"], tl["hsum"], tl["p1"]
                st6, mv, stat2, mvf = tl["st6"], tl["mv"], tl["stat2"], tl["mvf"]
                var_t, rstd, v2, rstd2 = (tl["var_t"], tl["rstd"], tl["v2"],
                                          tl["rstd2"])
                ddr, cxwl, so_t, rr = (tl["ddr"], tl["cxwl"], tl["so_t"],
                                       tl["rr"])
                if not fast_ln:
                    hnew = tl["hnew"]
                    st6b, mvb, stat2b = tl["st6b"], tl["mvb"], tl["stat2b"]
                    mvfb, var2_t = tl["mvfb"], tl["var2_t"]

                # stage x_aug.T column block for this step (fixed addr for LDW)
                nc.vector.tensor_copy(sx, xtb[:, bass.ds(x_off * 32, 32)])

                # ---- gate matmuls, column-tiled 128x32 mode ----
                # stationary chunks: sx [65,32], teT quarters, zT quarters
                stats = [(sx, wm0)] + \
                    [(teT[:, 32 * k:32 * k + 32],
                      wmr[:, D_GATE * k:D_GATE * (k + 1)]) for k in range(4)] + \
                    [(zT[:, 32 * k:32 * k + 32],
                      wmr[:, D_GATE * (4 + k):D_GATE * (5 + k)]) for k in range(4)]
                nk = len(stats)
                for k, (st_ap, mv_ap) in enumerate(stats):
                    for j in range(4):
                        nc.tensor.matmul(
                            ps_g[32 * j:32 * j + 32, :],
                            lhsT=st_ap,
                            rhs=mv_ap[:, 512 * j:512 * j + 512],
                            start=(k == 0), stop=(k == nk - 1),
                            tile_position=(0, 32 * j),
                            skip_group_check=True,
                        )

                # ---- gate activations (PSUM -> SBUF) ----
                # sigmoid(x) = 0.5*(1 + tanh(x/2)): the Tanh table is ~10x
                # more accurate than the Sigmoid table on this HW, and the
                # recurrence chaotically amplifies per-step noise.
                nc.scalar.activation(gates[:, 0:256], ps_g[:, 0:256],
                                     AF.Tanh, scale=0.5)
                nc.vector.tensor_scalar(out=gates[:, 0:256],
                                        in0=gates[:, 0:256], scalar1=0.5,
                                        scalar2=0.5, op0=OP.mult, op1=OP.add)
                nc.scalar.activation(gates[:, 256:384], ps_g[:, 256:384], AF.Tanh)
                nc.scalar.activation(gates[:, 384:512], ps_g[:, 384:512],
                                     AF.Tanh, scale=0.5)
                nc.vector.tensor_scalar(out=gates[:, 384:512],
                                        in0=gates[:, 384:512], scalar1=0.5,
                                        scalar2=0.5, op0=OP.mult, op1=OP.add)

                # ---- c update ----
                nc.vector.tensor_mul(tmp_ig, gates[:, 0:HQ], gates[:, 256:384])
                nc.vector.tensor_mul(c_r, gates[:, HQ:256], c_r)
                nc.vector.tensor_add(c_r, c_r, tmp_ig)

                # ---- u = o*tanh(c) + h ----
                nc.scalar.activation(tc_t, c_r, AF.Tanh)
                nc.vector.tensor_mul(u_t, gates[:, 384:512], tc_t)
                nc.vector.tensor_add(u_t, u_t, h_r)

                # ---- LN1 stats (mean/var over full H via quarter stats) ----
                nc.vector.bn_stats(out=st6, in_=u_t)
                nc.vector.bn_aggr(out=mv, in_=st6)
                # stat2 = [m_q, v_q + m_q^2]
                nc.vector.tensor_scalar(out=stat2[:, 1:2], in0=mv[:, 0:1],
                                        scalar1=mv[:, 0:1], scalar2=None,
                                        op0=OP.mult)
                nc.vector.tensor_add(stat2[:, 1:2], stat2[:, 1:2], mv[:, 1:2])
                nc.vector.tensor_copy(stat2[:, 0:1], mv[:, 0:1])
                # cross-quarter combine: ps_s = ssel.T @ stat2 (sums 4 blocks)
                nc.tensor.matmul(ps_s, lhsT=ssel, rhs=stat2, start=True, stop=True)
                nc.vector.tensor_scalar(out=mvf, in0=ps_s, scalar1=0.25,
                                        scalar2=None, op0=OP.mult)
                nc.vector.tensor_scalar(out=var_t, in0=mvf[:, 0:1],
                                        scalar1=mvf[:, 0:1], scalar2=None,
                                        op0=OP.mult)
                nc.vector.tensor_sub(var_t, mvf[:, 1:2], var_t)
                rsqrt_acc(tl, rstd, var_t)

                # ---- z1 = (u - m) * rstd;  h_next ----
                nc.vector.tensor_scalar(out=z1, in0=u_t, scalar1=mvf[:, 0:1],
                                        scalar2=rstd, op0=OP.subtract,
                                        op1=OP.mult)
                if fast_ln:
                    h_next = z1
                else:
                    nc.vector.tensor_mul(hnew, z1, gresh)
                    nc.vector.tensor_add(hnew, hnew, bresh)
                    h_next = hnew

                # hsum = h_next + h_old ; then h_r <- h_next
                nc.vector.tensor_add(hsum, h_next, h_r)
                nc.vector.tensor_copy(h_r, h_next)

                # ---- LN2 -> z2 (h_norm pre-affine; affine folded in weights) ----
                if fast_ln:
                    # h_next == z1: mean=0; var = rstd^2 * var
                    nc.vector.tensor_scalar(out=v2, in0=rstd, scalar1=rstd,
                                            scalar2=None, op0=OP.mult)
                    nc.vector.tensor_mul(v2, v2, var_t)
                    rsqrt_acc(tl, rstd2, v2)
                    nc.vector.tensor_scalar(out=z2, in0=z1, scalar1=rstd2,
                                            scalar2=None, op0=OP.mult)
                else:
                    nc.vector.bn_stats(out=st6b, in_=h_next)
                    nc.vector.bn_aggr(out=mvb, in_=st6b)
                    nc.vector.tensor_scalar(out=stat2b[:, 1:2], in0=mvb[:, 0:1],
                                            scalar1=mvb[:, 0:1], scalar2=None,
                                            op0=OP.mult)
                    nc.vector.tensor_add(stat2b[:, 1:2], stat2b[:, 1:2],
                                         mvb[:, 1:2])
                    nc.vector.tensor_copy(stat2b[:, 0:1], mvb[:, 0:1])
                    nc.tensor.matmul(ps_s2, lhsT=ssel, rhs=stat2b,
                                     start=True, stop=True)
                    nc.vector.tensor_scalar(out=mvfb, in0=ps_s2, scalar1=0.25,
                                            scalar2=None, op0=OP.mult)
                    nc.vector.tensor_scalar(out=var2_t, in0=mvfb[:, 0:1],
                                            scalar1=mvfb[:, 0:1], scalar2=None,
                                            op0=OP.mult)
                    nc.vector.tensor_sub(var2_t, mvfb[:, 1:2], var2_t)
                    rsqrt_acc(tl, rstd2, var2_t)
                    nc.vector.tensor_scalar(out=z2, in0=h_next,
                                            scalar1=mvfb[:, 0:1], scalar2=rstd2,
                                            op0=OP.subtract, op1=OP.mult)

                # ---- h_en update (resh) ----
                nc.vector.tensor_scalar(out=ddr, in0=cxc[:, bass.ds(t_g, 1)],
                                        scalar1=cxp[:, bass.ds(t_g, 1)],
                                        scalar2=0.5, op0=OP.subtract,
                                        op1=OP.mult)
                nc.vector.tensor_scalar(out=hsum, in0=hsum, scalar1=ddr,
                                        scalar2=None, op0=OP.mult)
                nc.vector.tensor_add(hen_r, hen_r, hsum)

                # ---- transposes for next step: zT = z2.T, heT = h_en.T ----
                # (columns of the full [128,128] transpose are ordered
                # 32q+b, so zT[:, 32k:32k+32] is exactly z2.T chunk k)
                nc.tensor.transpose(ps_tz, in_=z2, identity=id128)
                nc.tensor.transpose(ps_th, in_=hen_r, identity=id128)
                nc.scalar.activation(zT, ps_tz, AF.Copy)
                nc.scalar.activation(heT, ps_th, AF.Copy)

                # ---- t_en for next step: t_en.T = W_e @ h_en.T ----
                for m in range(4):
                    for kc in range(4):
                        nc.tensor.matmul(
                            ps_te[m],
                            lhsT=wet[:, (kc * 4 + m) * 128:(kc * 4 + m + 1) * 128],
                            rhs=heT[:, 32 * kc:32 * kc + 32],
                            start=(kc == 0), stop=(kc == 3),
                        )
                    nc.scalar.activation(teT[:, 32 * m:32 * m + 32],
                                         ps_te[m], AF.Tanh)

                # ---- output head: out = h_next . wfc + cur_x * wl ----
                nc.vector.tensor_mul(p1, h_next, wfcr)
                nc.vector.tensor_reduce(out=rr, in_=p1, axis=AX.X, op=OP.add)
                nc.tensor.matmul(ps_w, lhsT=ssel, rhs=rr, start=True, stop=True)
                nc.vector.tensor_scalar(out=cxwl, in0=cxc[:, bass.ds(t_g, 1)],
                                        scalar1=float(wl), scalar2=None,
                                        op0=OP.mult)
                nc.vector.tensor_scalar(out=outb[:, bass.ds(t_g + 1, 1)],
                                        in0=ps_w, scalar1=cxwl, scalar2=None,
                                        op0=OP.add)

                # ---- energy update ----
                nc.vector.tensor_add(so_t, outb[:, bass.ds(t_g + 1, 1)],
                                     outb[:, bass.ds(t_g, 1)])
                nc.vector.tensor_scalar(out=so_t, in0=so_t, scalar1=ddr,
                                        scalar2=None, op0=OP.mult)
                nc.vector.tensor_add(enb[:, bass.ds(t_g + 1, 1)],
                                     enb[:, bass.ds(t_g, 1)], so_t)

            def one_pass():
                init_state()
                for ch in range(n_chunks):
                    global xtb
                    xtb = xbufp.tile([I + 1, 32 * chunk], f32, tag="xtb")
                    nc.sync.dma_start(
                        out=xtb,
                        in_=d_xt[:, 32 * chunk * ch:32 * chunk * (ch + 1)])
                    if chunk == 1:
                        step_body(ch, 0)
                    elif chunk <= unroll:
                        for j2 in range(chunk):
                            step_body(ch * chunk + j2, j2)
                    else:
                        assert chunk % unroll == 0
                        with tc.For_i(0, chunk // unroll,
                                      staggered_reset=stagger) as iv:
                            for j2 in range(unroll):
                                step_body(ch * chunk + iv * unroll + j2,
                                          iv * unroll + j2)

            if repeats == 1:
                one_pass()
            else:
                with tc.For_i(0, repeats):
                    one_pass()

            # ---- final outputs ----
            nc.sync.dma_start(out=d_outs, in_=outb[0:BC, 1:t_total + 1])
            nc.sync.dma_start(out=d_ens, in_=enb[0:BC, 1:t_total + 1])
            nc.sync.dma_start(out=d_hf, in_=h_r)
            nc.sync.dma_start(out=d_cf, in_=c_r)
            nc.sync.dma_start(out=d_henf, in_=hen_r)

    nc.compile()
    return nc


def _prep_weights(W_e, W_i, b_i, W_f, b_f, W_c, b_c, W_o, b_o,
                  ln_g, ln_b, W_fc, fast_ln, wet_bf16=True):
    """Host-side weight preprocessing (shared across cores)."""
    import ml_dtypes

    # Gate output permutation: column n' = q*512 + g*128 + j corresponds to
    # gate g (i,f,c,o), H index 128*q + j.
    Wg = np.stack([W_i, W_f, W_c, W_o])            # [4, 512, 1088]
    bg = np.stack([b_i, b_f, b_c, b_o])            # [4, 512]
    # split K columns: x (0:64), t_en (64:576), h (576:1088)
    Wx = Wg[:, :, :I]                              # [4, 512, 64]
    Wt = Wg[:, :, I:I + H]                         # [4, 512, 512]
    Wh = Wg[:, :, I + H:]                          # [4, 512, 512]
    # fold LN affine of h_norm into Wh / bias
    Whp = Wh * ln_g[None, None, :]
    bias = bg + Wh @ ln_b                          # [4, 512]

    def perm_out(M):
        # [4, 512, K] -> [2048(K-order for moving) ...] -> permuted [K, 2048]
        # output order: for q in 4: for g in 4: 128 cols of gate g quarter q
        M = M.reshape(4, Q, HQ, -1)                # [g, q, j, K]
        M = M.transpose(1, 0, 2, 3).reshape(D_GATE, -1)  # [q*g*j, K]
        return M

    Wx_p = perm_out(Wx)                            # [2048, 64]
    Wt_p = perm_out(Wt)                            # [2048, 512]
    Wh_p = perm_out(Whp)                           # [2048, 512]
    bias_p = perm_out(bias[:, :, None])[:, 0]      # [2048]

    wm0 = np.concatenate([Wx_p.T, bias_p[None, :]], axis=0)  # [65, 2048]
    # moving chunks 0..3: t_en.T K-chunks; 4..7: z2.T K-chunks
    wmr = np.empty((128, 8 * D_GATE), np.float32)
    for k in range(4):
        wmr[:, D_GATE * k:D_GATE * (k + 1)] = Wt_p.T[128 * k:128 * (k + 1), :]
    for k in range(4):
        wmr[:, D_GATE * (4 + k):D_GATE * (5 + k)] = \
            Wh_p.T[128 * k:128 * (k + 1), :]

    # W_e.T chunks for t_en.T = W_e @ h_en.T: lhsT block (kc, m) is
    # W_e.T[128*kc:.., 128*m:..] stored at cols (kc*4+m)*128.
    WeT = W_e.T.astype(np.float32)                 # [512(k), 512(m)]
    wet = np.empty((128, 2048), np.float32)
    for kc in range(4):
        for m in range(4):
            wet[:, (kc * 4 + m) * 128:(kc * 4 + m + 1) * 128] = \
                WeT[128 * kc:128 * (kc + 1), 128 * m:128 * (m + 1)]
    if wet_bf16:
        wet = wet.astype(ml_dtypes.bfloat16)

    # wfc resh const [128, HQ]
    wfc_h = W_fc[0, :H].astype(np.float32)
    wfcr = np.broadcast_to(
        wfc_h.reshape(Q, 1, HQ), (Q, BC, HQ)).reshape(128, HQ).copy()
    wl = float(W_fc[0, H])

    # selector matrix S[p, p2] = 1 if p % 32 == p2 % 32
    p = np.arange(128)
    ssel = (p[:, None] % 32 == p[None, :] % 32).astype(np.float32)
    id128 = np.eye(128, dtype=np.float32)

    out = dict(wm0=wm0.astype(np.float32), wmr=wmr, wet=wet,
               wfcr=wfcr, ssel=ssel, id128=id128, wl=wl)
    if not fast_ln:
        out["gresh"] = np.broadcast_to(
            ln_g.reshape(Q, 1, HQ), (Q, BC, HQ)).reshape(128, HQ).copy()
        out["bresh"] = np.broadcast_to(
            ln_b.reshape(Q, 1, HQ), (Q, BC, HQ)).reshape(128, HQ).copy()
    return out


def _prep_core_inputs(x_core, wdict, fast_ln, t_total):
    """Per-core input dict. x_core: [BC, T, I]."""
    bc = x_core.shape[0]
    # x_aug.T: [65, t*32 + b]; row 64 = ones (bias row)
    xt = np.empty((I + 1, bc * t_total), np.float32)
    xt[:I] = x_core.transpose(2, 1, 0).reshape(I, t_total * bc)
    xt[I] = 1.0
    cx = x_core[:, :, 0].astype(np.float32)        # [BC, T]
    cxc = np.tile(cx, (4, 1))                      # [128, T] resh-replicated
    cxp = np.zeros_like(cx)
    cxp[:, 1:] = cx[:, :-1]
    cxp = np.tile(cxp, (4, 1))
    d = dict(xt=xt, cxc=cxc, cxp=cxp,
             wm0=wdict["wm0"], wmr=wdict["wmr"], wet=wdict["wet"],
             wfcr=wdict["wfcr"], ssel=wdict["ssel"], id128=wdict["id128"])
    if not fast_ln:
        d["gresh"] = wdict["gresh"]
        d["bresh"] = wdict["bresh"]
    return d


def kernel(x, W_e, W_i, b_i, W_f, b_f, W_c, b_c, W_o, b_o, ln_g, ln_b, W_fc,
           n_chunks=8, chunk=128, repeats=1, return_results=True,
           wet_bf16=True, unroll=1, stagger=False, _bench_out=None):
    from concourse.bass_utils import run_bass_kernel_spmd

    args = [np.asarray(a, np.float32 if np.asarray(a).dtype != np.int32
                       else np.int32) for a in
            (x, W_e, W_i, b_i, W_f, b_f, W_c, b_c, W_o, b_o, ln_g, ln_b, W_fc)]
    (x, W_e, W_i, b_i, W_f, b_f, W_c, b_c, W_o, b_o, ln_g, ln_b, W_fc) = args

    t_total = n_chunks * chunk
    fast_ln = bool(np.allclose(ln_g, 1.0) and np.allclose(ln_b, 0.0))

    wdict = _prep_weights(W_e, W_i, b_i, W_f, b_f, W_c, b_c, W_o, b_o,
                          ln_g, ln_b, W_fc, fast_ln, wet_bf16)

    key = (n_chunks, chunk, fast_ln, round(wdict["wl"], 12), repeats,
           wet_bf16, unroll, stagger)
    if key not in _PROGRAM_CACHE:
        _PROGRAM_CACHE[key] = _build_program(
            n_chunks, chunk, fast_ln, wdict["wl"], repeats, wet_bf16,
            unroll, stagger)
    nc = _PROGRAM_CACHE[key]

    in_maps = []
    for c in range(N_CORES):
        x_core = x[BC * c:BC * (c + 1), :t_total]
        in_maps.append(_prep_core_inputs(x_core, wdict, fast_ln, t_total))

    import time
    t0 = time.perf_counter()
    res = run_bass_kernel_spmd(nc, in_maps, core_ids=list(range(N_CORES)))
    wall = time.perf_counter() - t0
    if _bench_out is not None:
        _bench_out["wall_s"] = wall

    if not return_results:
        return None

    outputs = np.empty((B, t_total, 1), np.float32)
    energies = np.empty((B, t_total, 1), np.float32)
    h_f = np.empty((B, H), np.float32)
    c_f = np.empty((B, H), np.float32)
    hen_f = np.empty((B, H), np.float32)
    for c in range(N_CORES):
        r = res.results[c]
        sl = slice(BC * c, BC * (c + 1))
        outputs[sl, :, 0] = r["outs"]
        energies[sl, :, 0] = r["ens"]
        h_f[sl] = _unresh(r["hf"])
        c_f[sl] = _unresh(r["cf"])
        hen_f[sl] = _unresh(r["henf"])

    prev_out = outputs[:, t_total - 1, :].copy()       # [B, 1]
    en_f = energies[:, t_total - 1, :].copy()          # [B, 1]
    prev_x = x[:, t_total - 1, :1].copy()              # [B, 1]
    carry = (h_f, c_f, prev_out, hen_f, en_f, prev_x)
    return outputs, energies, carry
